# revision 11
# baseline (speedup 1.0000x reference)
"""Expert-parallel MoE kernel for Trainium2 (8 NeuronCores, Bass/Tile).

Sharding: expert dim E=256 split 32-per-core across 8 cores; router is
evaluated on the host (128x256 — negligible) and each core receives its
local experts' weights plus the per-token combine weights for those
experts. Each core computes the combine-weighted partial output of its
32 experts; the host sums the 8 partials. No device collectives needed.

Per-core device pipeline (per expert e):
  DMA w1[e] (2MB), w2[e] (2MB), b1[e]
  h = x @ w1[e] + b1[e]        4+1 matmuls x 2 PSUM halves (xT stationary)
  hg = gelu(h)                 ScalarE erf-GELU, PSUM -> SBUF
  hg *= comb[:, e]             VectorE per-partition scale (folds top-k weight)
  hT = transpose(hg)           8x PE transpose, -> PSUM -> SBUF
  y += hT.T @ w2[e]            8 matmuls accumulating one PSUM bank
finally y += comb_local @ b2_local (one K=32 matmul), DMA out.
"""

import numpy as np

B, T, DIM = 2, 64, 512
E, H, K = 256, 1024, 42
N = B * T                     # 128 tokens
N_CORES = 8
EPC = E // N_CORES            # 32 experts per core

# matmul dtype: "float32r" = fp32 bits, relaxed-precision PE mode (1 cyc/row
# at N>=256 vs 4 for exact fp32). Flip to "float32" if accuracy demands.
MM_DTYPE = "float32r"

_prog_cache = {}


def _build_program(mm_dtype_name, act="Gelu"):
    from contextlib import ExitStack

    import concourse.bacc as bacc
    import concourse.mybir as mybir
    import concourse.tile as tile

    f32 = mybir.dt.float32
    # All matmul operands are declared in the matmul dtype end-to-end (the
    # BIR verifier requires fp32r consumers to see fp32r producers). For
    # float32r the bits are plain fp32 on the host side.
    mdt = getattr(mybir.dt, mm_dtype_name)
    GELU = getattr(mybir.ActivationFunctionType, act)

    KD = DIM // 128          # 4 contraction slices for x @ w1
    KH = H // 128            # 8 contraction slices for h @ w2
    NSEG = H // 512          # 2 PSUM halves for h

    nc = bacc.Bacc("TRN2", target_bir_lowering=False, debug=False,
                   num_devices=N_CORES)

    xT_d = nc.dram_tensor("xT", [DIM, N], mdt, kind="ExternalInput")
    w1_d = nc.dram_tensor("w1s", [EPC, DIM, H], mdt, kind="ExternalInput")
    b1_d = nc.dram_tensor("b1s", [EPC, H], mdt, kind="ExternalInput")
    w2_d = nc.dram_tensor("w2s", [EPC, H, DIM], mdt, kind="ExternalInput")
    b2_d = nc.dram_tensor("b2s", [EPC, DIM], mdt, kind="ExternalInput")
    cc_d = nc.dram_tensor("combc", [N, EPC], f32, kind="ExternalInput")
    ct_d = nc.dram_tensor("combT", [EPC, N], mdt, kind="ExternalInput")
    id_d = nc.dram_tensor("ident", [128, 128], f32, kind="ExternalInput")
    ones_d = nc.dram_tensor("ones", [1, N], mdt, kind="ExternalInput")
    out_d = nc.dram_tensor("out", [N, DIM], f32, kind="ExternalOutput")

    with tile.TileContext(nc) as tc, ExitStack() as ctx:
        const = ctx.enter_context(tc.tile_pool(name="const", bufs=1))
        w1p = ctx.enter_context(tc.tile_pool(name="w1p", bufs=3))
        w2p = ctx.enter_context(tc.tile_pool(name="w2p", bufs=3))
        b1p = ctx.enter_context(tc.tile_pool(name="b1p", bufs=2))
        hgp = ctx.enter_context(tc.tile_pool(name="hgp", bufs=2))
        hTsp = ctx.enter_context(tc.tile_pool(name="hTsp", bufs=2))
        outp = ctx.enter_context(tc.tile_pool(name="outp", bufs=1))
        hps = ctx.enter_context(tc.tile_pool(name="hps", bufs=2, space="PSUM"))
        hTps = ctx.enter_context(tc.tile_pool(name="hTps", bufs=1, space="PSUM"))
        yps = ctx.enter_context(tc.tile_pool(name="yps", bufs=1, space="PSUM"))

        xT_sb = const.tile([128, KD * N], mdt)
        nc.sync.dma_start(
            xT_sb[:].rearrange("p (k t) -> p k t", k=KD),
            xT_d[:, :].rearrange("(k p) t -> p k t", p=128),
        )
        id_sb = const.tile([128, 128], f32)
        nc.sync.dma_start(id_sb[:], id_d[:, :])
        cc_sb = const.tile([N, EPC], f32)
        nc.sync.dma_start(cc_sb[:], cc_d[:, :])
        ct_sb = const.tile([EPC, N], mdt)
        nc.sync.dma_start(ct_sb[:], ct_d[:, :])
        b2_sb = const.tile([EPC, DIM], mdt)
        nc.sync.dma_start(b2_sb[:], b2_d[:, :])
        ones_sb = const.tile([1, N], mdt)
        nc.sync.dma_start(ones_sb[:], ones_d[:, :])

        y_ps = yps.tile([N, DIM], f32)

        for e in range(EPC):
            w1_t = w1p.tile([128, KD * H], mdt)
            nc.sync.dma_start(
                w1_t[:].rearrange("p (k h) -> p k h", k=KD),
                w1_d[e].rearrange("(k p) h -> p k h", p=128),
            )
            w2_t = w2p.tile([128, KH * DIM], mdt)
            nc.sync.dma_start(
                w2_t[:].rearrange("p (k d) -> p k d", k=KH),
                w2_d[e].rearrange("(k p) d -> p k d", p=128),
            )
            b1_t = b1p.tile([1, H], mdt)
            nc.sync.dma_start(b1_t[:], b1_d[e : e + 1, :])

            h_ps = hps.tile([N, H], f32)
            for s in range(NSEG):
                seg = slice(s * 512, (s + 1) * 512)
                for k in range(KD):
                    nc.tensor.matmul(
                        h_ps[:, seg],
                        lhsT=xT_sb[:, k * N : (k + 1) * N],
                        rhs=w1_t[:, k * H + s * 512 : k * H + s * 512 + 512],
                        start=(k == 0), stop=False,
                    )
                nc.tensor.matmul(
                    h_ps[:, seg],
                    lhsT=ones_sb[:],
                    rhs=b1_t[:, seg],
                    start=False, stop=True,
                )

            hg = hgp.tile([N, H], f32)
            nc.scalar.activation(hg[:], h_ps[:], GELU)
            nc.vector.tensor_scalar_mul(hg[:], hg[:], cc_sb[:, e : e + 1])

            hT_ps = hTps.tile([128, H], f32)
            for j in range(KH):
                nc.tensor.transpose(
                    hT_ps[:, j * 128 : (j + 1) * 128],
                    hg[:, j * 128 : (j + 1) * 128],
                    id_sb[:],
                )
            hT_sb = hTsp.tile([128, H], mdt)
            nc.vector.tensor_copy(hT_sb[:], hT_ps[:])

            for j in range(KH):
                nc.tensor.matmul(
                    y_ps[:],
                    lhsT=hT_sb[:, j * 128 : (j + 1) * 128],
                    rhs=w2_t[:, j * DIM : (j + 1) * DIM],
                    start=(e == 0 and j == 0), stop=False,
                )

        nc.tensor.matmul(
            y_ps[:], lhsT=ct_sb[:], rhs=b2_sb[:],
            start=False, stop=True,
        )
        o_sb = outp.tile([N, DIM], f32)
        nc.vector.tensor_copy(o_sb[:], y_ps[:])
        nc.sync.dma_start(out_d[:, :], o_sb[:])

    nc.compile()
    return nc


def get_program(mm_dtype_name=MM_DTYPE, act="Gelu"):
    key = (mm_dtype_name, act)
    if key not in _prog_cache:
        _prog_cache[key] = _build_program(mm_dtype_name, act)
    return _prog_cache[key]


def _softmax(v, axis=-1):
    m = np.max(v, axis=axis, keepdims=True)
    ex = np.exp(v - m)
    return ex / np.sum(ex, axis=axis, keepdims=True)


def host_routing(x, router_w, router_b):
    """Replicates the reference routing in fp32 numpy: softmax over all
    experts, take top-K probs, renormalize those with another softmax."""
    xt = np.asarray(x, np.float32).reshape(N, DIM)
    logits = xt @ np.asarray(router_w, np.float32) + np.asarray(router_b, np.float32)
    probs = _softmax(logits, axis=-1)
    idx = np.argpartition(probs, E - K, axis=-1)[:, E - K:]          # top-K set
    vals = np.take_along_axis(probs, idx, axis=-1)
    w = _softmax(vals, axis=-1)
    comb = np.zeros((N, E), np.float32)
    np.put_along_axis(comb, idx, w.astype(np.float32), axis=-1)
    return comb


def make_in_maps(x, w1, b1, w2, b2, router_w, router_b):
    x = np.ascontiguousarray(np.asarray(x, np.float32))
    w1 = np.asarray(w1, np.float32)
    b1 = np.asarray(b1, np.float32)
    w2 = np.asarray(w2, np.float32)
    b2 = np.asarray(b2, np.float32)
    comb = host_routing(x, router_w, router_b)
    xT = np.ascontiguousarray(x.reshape(N, DIM).T)
    ident = np.eye(128, dtype=np.float32)
    in_maps = []
    for c in range(N_CORES):
        sl = slice(c * EPC, (c + 1) * EPC)
        cl = np.ascontiguousarray(comb[:, sl])
        in_maps.append({
            "xT": xT,
            "w1s": np.ascontiguousarray(w1[sl]),
            "b1s": np.ascontiguousarray(b1[sl]),
            "w2s": np.ascontiguousarray(w2[sl]),
            "b2s": np.ascontiguousarray(b2[sl]),
            "combc": cl,
            "combT": np.ascontiguousarray(cl.T),
            "ident": ident,
            "ones": np.ones((1, N), np.float32),
        })
    return in_maps


def kernel(x, w1, b1, w2, b2, router_w, router_b):
    from concourse.bass_utils import run_bass_kernel_spmd

    nc = get_program()
    in_maps = make_in_maps(x, w1, b1, w2, b2, router_w, router_b)
    res = run_bass_kernel_spmd(nc, in_maps, list(range(N_CORES)))
    out = np.zeros((N, DIM), np.float32)
    for r in res.results:
        out += r["out"]
    return out.reshape(B, T, DIM).astype(np.float32)


# revision 34
# speedup vs baseline: 242.2354x; 242.2354x over previous
"""Expert-parallel MoE kernel for Trainium2 (8 NeuronCores, Bass/Tile).

Sharding: expert dim E=256 split 32-per-core across 8 cores; router is
evaluated on the host (128x256 — negligible) and each core receives its
local experts' weights plus the per-token combine weights for those
experts. Each core computes the combine-weighted partial output of its
32 experts; the host sums the 8 partials. No device collectives needed.

Weights are host-packed per expert PAIR into one contiguous block
[128 partitions x 16384 fp32] holding w1 (pre-tiled [i, k, h]) then w2
([i, k, d]); each pair streams as 4 perfectly-linear 2MB DMAs on the SP
HWDGE ring (b1 rides the gpsimd SWDGE ring so the weight stream never
stalls), measured ~97% of single-core HBM line rate.
Matmuls use float32r (fp32 bits, relaxed PE mode,
1 cyc/row) with x-transposed as the stationary operand so the streamed
weights are the moving operand. Per expert: h matmuls -> erf-GELU on
ScalarE (bias added via ones-row matmul) -> fold top-k combine weight in
with a per-partition VectorE scale -> PE transpose -> second matmul
accumulating all experts into one PSUM bank; + one K=32 matmul for the
b2 term; single output DMA.
"""

import numpy as np

B, T, DIM = 2, 64, 512
E, H, K = 256, 1024, 42
N = B * T                     # 128 tokens
N_CORES = 8
EPC = E // N_CORES            # 32 experts per core
GP = EPC // 2                 # 16 expert pairs per core

# fp32 bits, relaxed-precision PE mode (1 cyc/row at N>=256 vs 4 for exact
# fp32). Flip to "float32" if accuracy demands.
MM_DTYPE = "float32r"

W1B = 4 * H                   # fp32 elements of one expert's w1 per partition
W2B = 8 * DIM                 # fp32 elements of one expert's w2 per partition
PAIRW = 2 * (W1B + W2B)       # 16384 elements per partition per pair

_prog_cache = {}


def _build_program(mm_dtype_name, act="Gelu", n_pairs=GP, repeat=1,
                   wsplit=4, rings=("sync",), group=2, wbufs=2,
                   b1eng="gpsimd"):
    from contextlib import ExitStack

    import concourse.bacc as bacc
    import concourse.mybir as mybir
    import concourse.tile as tile

    f32 = mybir.dt.float32
    # Matmul operands are declared in the matmul dtype end-to-end (the BIR
    # verifier requires fp32r consumers to see fp32r producers). For
    # float32r the bits are plain fp32 on the host side.
    mdt = getattr(mybir.dt, mm_dtype_name)
    GELU = getattr(mybir.ActivationFunctionType, act)

    KD = DIM // 128          # 4 contraction slices for x @ w1
    KH = H // 128            # 8 contraction slices for h @ w2
    NSEG = H // 512          # 2 PSUM halves for h

    nc = bacc.Bacc("TRN2", target_bir_lowering=False, debug=False,
                   num_devices=N_CORES)

    xT_d = nc.dram_tensor("xT", [DIM, N], mdt, kind="ExternalInput")
    n_grp = EPC // group
    grpw = group * (W1B + W2B)
    wpk_d = nc.dram_tensor("wpk", [n_grp, 128, grpw], mdt, kind="ExternalInput")
    b1_d = nc.dram_tensor("b1s", [EPC, H], mdt, kind="ExternalInput")
    b2_d = nc.dram_tensor("b2s", [EPC, DIM], mdt, kind="ExternalInput")
    cc_d = nc.dram_tensor("combc", [N, EPC], f32, kind="ExternalInput")
    ct_d = nc.dram_tensor("combT", [EPC, N], mdt, kind="ExternalInput")
    id_d = nc.dram_tensor("ident", [128, 128], f32, kind="ExternalInput")
    ones_d = nc.dram_tensor("ones", [1, N], mdt, kind="ExternalInput")
    out_d = nc.dram_tensor("out", [N, DIM], f32, kind="ExternalOutput")

    with tile.TileContext(nc) as tc, ExitStack() as ctx:
        const = ctx.enter_context(tc.tile_pool(name="const", bufs=1))
        wp = ctx.enter_context(tc.tile_pool(name="wp", bufs=wbufs))
        b1p = ctx.enter_context(tc.tile_pool(name="b1p", bufs=2))
        hgp = ctx.enter_context(tc.tile_pool(name="hgp", bufs=2))
        hTsp = ctx.enter_context(tc.tile_pool(name="hTsp", bufs=2))
        outp = ctx.enter_context(tc.tile_pool(name="outp", bufs=1))
        hps = ctx.enter_context(tc.tile_pool(name="hps", bufs=2, space="PSUM"))
        hTps = ctx.enter_context(tc.tile_pool(name="hTps", bufs=1, space="PSUM"))
        yps = ctx.enter_context(tc.tile_pool(name="yps", bufs=1, space="PSUM"))

        xT_sb = const.tile([128, KD * N], mdt)
        nc.sync.dma_start(
            xT_sb[:].rearrange("p (k t) -> p k t", k=KD),
            xT_d[:, :].rearrange("(k p) t -> p k t", p=128),
        )
        id_sb = const.tile([128, 128], f32)
        nc.sync.dma_start(id_sb[:], id_d[:, :])
        cc_sb = const.tile([N, EPC], f32)
        nc.sync.dma_start(cc_sb[:], cc_d[:, :])
        ct_sb = const.tile([EPC, N], mdt)
        nc.sync.dma_start(ct_sb[:], ct_d[:, :])
        b2_sb = const.tile([EPC, DIM], mdt)
        nc.sync.dma_start(b2_sb[:], b2_d[:, :])
        ones_sb = const.tile([1, N], mdt)
        nc.sync.dma_start(ones_sb[:], ones_d[:, :])

        y_ps = yps.tile([N, DIM], f32)

        def emit_experts():
            for g in range(n_grp):
                w_t = wp.tile([128, grpw], mdt)
                csz = grpw // wsplit
                for ci in range(wsplit):
                    eng = getattr(nc, rings[ci % len(rings)])
                    eng.dma_start(w_t[:, ci * csz : (ci + 1) * csz],
                                  wpk_d[g][:, ci * csz : (ci + 1) * csz])
                b1_t = b1p.tile([1, group * H], mdt)
                getattr(nc, b1eng).dma_start(
                    b1_t[:].rearrange("o (i h) -> o i h", i=group),
                    b1_d[group * g : group * (g + 1), :].rearrange(
                        "(o i) h -> o i h", o=1),
                )
                for i in range(group):
                    e = group * g + i
                    h_ps = hps.tile([N, H], f32)
                    for s in range(NSEG):
                        seg = slice(s * 512, (s + 1) * 512)
                        for k in range(KD):
                            nc.tensor.matmul(
                                h_ps[:, seg],
                                lhsT=xT_sb[:, k * N : (k + 1) * N],
                                rhs=w_t[:, i * W1B + k * H + s * 512 :
                                        i * W1B + k * H + s * 512 + 512],
                                start=(k == 0), stop=False,
                            )
                        nc.tensor.matmul(
                            h_ps[:, seg],
                            lhsT=ones_sb[:],
                            rhs=b1_t[:, i * H + s * 512 : i * H + (s + 1) * 512],
                            start=False, stop=True,
                        )

                    hg = hgp.tile([N, H], f32)
                    nc.scalar.activation(hg[:], h_ps[:], GELU)
                    nc.vector.tensor_scalar_mul(hg[:], hg[:], cc_sb[:, e : e + 1])

                    hT_ps = hTps.tile([128, H], f32)
                    for j in range(KH):
                        nc.tensor.transpose(
                            hT_ps[:, j * 128 : (j + 1) * 128],
                            hg[:, j * 128 : (j + 1) * 128],
                            id_sb[:],
                        )
                    hT_sb = hTsp.tile([128, H], mdt)
                    nc.vector.tensor_copy(hT_sb[:], hT_ps[:])

                    w2off = group * W1B + i * W2B
                    for j in range(KH):
                        nc.tensor.matmul(
                            y_ps[:],
                            lhsT=hT_sb[:, j * 128 : (j + 1) * 128],
                            rhs=w_t[:, w2off + j * DIM : w2off + (j + 1) * DIM],
                            start=(e == 0 and j == 0), stop=False,
                        )

        if repeat > 1:
            # timing-only variant: re-run the whole expert sweep on-device
            # to amortize host/tunnel dispatch overhead
            with tc.For_i(0, repeat, 1):
                emit_experts()
        else:
            emit_experts()

        nc.tensor.matmul(
            y_ps[:], lhsT=ct_sb[:], rhs=b2_sb[:],
            start=False, stop=True,
        )
        o_sb = outp.tile([N, DIM], f32)
        nc.vector.tensor_copy(o_sb[:], y_ps[:])
        nc.sync.dma_start(out_d[:, :], o_sb[:])

    nc.compile()
    return nc


def get_program(mm_dtype_name=MM_DTYPE, act="Gelu", n_pairs=GP, repeat=1,
                wsplit=4, rings=("sync",), group=2, wbufs=2,
                b1eng="gpsimd"):
    key = (mm_dtype_name, act, n_pairs, repeat, wsplit, tuple(rings), group,
           wbufs, b1eng)
    if key not in _prog_cache:
        _prog_cache[key] = _build_program(mm_dtype_name, act, n_pairs, repeat,
                                          wsplit, rings, group, wbufs, b1eng)
    return _prog_cache[key]


def _softmax(v, axis=-1):
    m = np.max(v, axis=axis, keepdims=True)
    ex = np.exp(v - m)
    return ex / np.sum(ex, axis=axis, keepdims=True)


def host_routing(x, router_w, router_b):
    """Replicates the reference routing in fp32 numpy: softmax over all
    experts, take top-K probs, renormalize those with another softmax."""
    xt = np.asarray(x, np.float32).reshape(N, DIM)
    logits = xt @ np.asarray(router_w, np.float32) + np.asarray(router_b, np.float32)
    probs = _softmax(logits, axis=-1)
    idx = np.argpartition(probs, E - K, axis=-1)[:, E - K:]          # top-K set
    vals = np.take_along_axis(probs, idx, axis=-1)
    w = _softmax(vals, axis=-1)
    comb = np.zeros((N, E), np.float32)
    np.put_along_axis(comb, idx, w.astype(np.float32), axis=-1)
    return comb


def pack_weights(w1c, w2c, group=2):
    """[32,512,1024] + [32,1024,512] -> [32/group, 128, group*12288]:
    per expert group, per partition, [w1(i,k,h) | w2(i,k,d)] contiguous."""
    ng = EPC // group
    a = (w1c.reshape(ng, group, KD_, 128, H).transpose(0, 3, 1, 2, 4)
         .reshape(ng, 128, group * W1B))
    b = (w2c.reshape(ng, group, KH_, 128, DIM).transpose(0, 3, 1, 2, 4)
         .reshape(ng, 128, group * W2B))
    return np.ascontiguousarray(np.concatenate([a, b], axis=2))


KD_ = DIM // 128
KH_ = H // 128


def make_in_maps(x, w1, b1, w2, b2, router_w, router_b, group=2):
    x = np.ascontiguousarray(np.asarray(x, np.float32))
    w1 = np.asarray(w1, np.float32)
    b1 = np.asarray(b1, np.float32)
    w2 = np.asarray(w2, np.float32)
    b2 = np.asarray(b2, np.float32)
    comb = host_routing(x, router_w, router_b)
    xT = np.ascontiguousarray(x.reshape(N, DIM).T)
    ident = np.eye(128, dtype=np.float32)
    in_maps = []
    for c in range(N_CORES):
        sl = slice(c * EPC, (c + 1) * EPC)
        cl = np.ascontiguousarray(comb[:, sl])
        in_maps.append({
            "xT": xT,
            "wpk": pack_weights(w1[sl], w2[sl], group),
            "b1s": np.ascontiguousarray(b1[sl]),
            "b2s": np.ascontiguousarray(b2[sl]),
            "combc": cl,
            "combT": np.ascontiguousarray(cl.T),
            "ident": ident,
            "ones": np.ones((1, N), np.float32),
        })
    return in_maps


def kernel(x, w1, b1, w2, b2, router_w, router_b):
    from concourse.bass_utils import run_bass_kernel_spmd

    nc = get_program()
    in_maps = make_in_maps(x, w1, b1, w2, b2, router_w, router_b)
    res = run_bass_kernel_spmd(nc, in_maps, list(range(N_CORES)))
    out = np.zeros((N, DIM), np.float32)
    for r in res.results:
        out += r["out"]
    return out.reshape(B, T, DIM).astype(np.float32)


# revision 35
# speedup vs baseline: 244.4983x; 1.0093x over previous
"""Expert-parallel MoE kernel for Trainium2 (8 NeuronCores, Bass/Tile).

Sharding: expert dim E=256 split 32-per-core across 8 cores; router is
evaluated on the host (128x256 — negligible) and each core receives its
local experts' weights plus the per-token combine weights for those
experts. Each core computes the combine-weighted partial output of its
32 experts; the host sums the 8 partials. No device collectives needed.

Weights are host-packed per expert PAIR into one contiguous block
[128 partitions x 16384 fp32] holding w1 (pre-tiled [i, k, h]) then w2
([i, k, d]); each pair streams as 4 perfectly-linear 2MB DMAs on the SP
HWDGE ring (b1 rides the gpsimd SWDGE ring so the weight stream never
stalls), measured ~97% of single-core HBM line rate.
Matmuls use float32r (fp32 bits, relaxed PE mode,
1 cyc/row) with x-transposed as the stationary operand so the streamed
weights are the moving operand. Per expert: h matmuls -> erf-GELU on
ScalarE (bias added via ones-row matmul) -> fold top-k combine weight in
with a per-partition VectorE scale -> PE transpose -> second matmul
accumulating all experts into one PSUM bank; + one K=32 matmul for the
b2 term; single output DMA.
"""

import numpy as np

B, T, DIM = 2, 64, 512
E, H, K = 256, 1024, 42
N = B * T                     # 128 tokens
N_CORES = 8
EPC = E // N_CORES            # 32 experts per core
GP = EPC // 2                 # 16 expert pairs per core

# fp32 bits, relaxed-precision PE mode (1 cyc/row at N>=256 vs 4 for exact
# fp32). Flip to "float32" if accuracy demands.
MM_DTYPE = "float32r"

W1B = 4 * H                   # fp32 elements of one expert's w1 per partition
W2B = 8 * DIM                 # fp32 elements of one expert's w2 per partition
PAIRW = 2 * (W1B + W2B)       # 16384 elements per partition per pair

_prog_cache = {}


def _build_program(mm_dtype_name, act="Gelu", n_pairs=GP, repeat=1,
                   wsplit=4, rings=("sync",), group=2, wbufs=2,
                   b1eng="gpsimd"):
    from contextlib import ExitStack

    import concourse.bacc as bacc
    import concourse.mybir as mybir
    import concourse.tile as tile

    f32 = mybir.dt.float32
    # Matmul operands are declared in the matmul dtype end-to-end (the BIR
    # verifier requires fp32r consumers to see fp32r producers). For
    # float32r the bits are plain fp32 on the host side.
    mdt = getattr(mybir.dt, mm_dtype_name)
    GELU = getattr(mybir.ActivationFunctionType, act)

    KD = DIM // 128          # 4 contraction slices for x @ w1
    KH = H // 128            # 8 contraction slices for h @ w2
    NSEG = H // 512          # 2 PSUM halves for h

    nc = bacc.Bacc("TRN2", target_bir_lowering=False, debug=False,
                   num_devices=N_CORES)

    xT_d = nc.dram_tensor("xT", [DIM, N], mdt, kind="ExternalInput")
    n_grp = EPC // group
    grpw = group * (W1B + W2B)
    wpk_d = nc.dram_tensor("wpk", [n_grp, 128, grpw], mdt, kind="ExternalInput")
    b1_d = nc.dram_tensor("b1s", [EPC, H], mdt, kind="ExternalInput")
    b2_d = nc.dram_tensor("b2s", [EPC, DIM], mdt, kind="ExternalInput")
    cc_d = nc.dram_tensor("combc", [N, EPC], f32, kind="ExternalInput")
    ct_d = nc.dram_tensor("combT", [EPC, N], mdt, kind="ExternalInput")
    id_d = nc.dram_tensor("ident", [128, 128], f32, kind="ExternalInput")
    ones_d = nc.dram_tensor("ones", [1, N], mdt, kind="ExternalInput")
    out_d = nc.dram_tensor("out", [N, DIM], f32, kind="ExternalOutput")

    with tile.TileContext(nc) as tc, ExitStack() as ctx:
        const = ctx.enter_context(tc.tile_pool(name="const", bufs=1))
        wp = ctx.enter_context(tc.tile_pool(name="wp", bufs=wbufs))
        b1p = ctx.enter_context(tc.tile_pool(name="b1p", bufs=2))
        hgp = ctx.enter_context(tc.tile_pool(name="hgp", bufs=2))
        hTsp = ctx.enter_context(tc.tile_pool(name="hTsp", bufs=2))
        outp = ctx.enter_context(tc.tile_pool(name="outp", bufs=1))
        hps = ctx.enter_context(tc.tile_pool(name="hps", bufs=2, space="PSUM"))
        hTps = ctx.enter_context(tc.tile_pool(name="hTps", bufs=1, space="PSUM"))
        yps = ctx.enter_context(tc.tile_pool(name="yps", bufs=1, space="PSUM"))

        xT_sb = const.tile([128, KD * N], mdt)
        nc.sync.dma_start(
            xT_sb[:].rearrange("p (k t) -> p k t", k=KD),
            xT_d[:, :].rearrange("(k p) t -> p k t", p=128),
        )
        id_sb = const.tile([128, 128], f32)
        nc.sync.dma_start(id_sb[:], id_d[:, :])
        cc_sb = const.tile([N, EPC], f32)
        nc.sync.dma_start(cc_sb[:], cc_d[:, :])
        ct_sb = const.tile([EPC, N], mdt)
        nc.sync.dma_start(ct_sb[:], ct_d[:, :])
        b2_sb = const.tile([EPC, DIM], mdt)
        nc.sync.dma_start(b2_sb[:], b2_d[:, :])
        ones_sb = const.tile([1, N], mdt)
        nc.sync.dma_start(ones_sb[:], ones_d[:, :])

        y_ps = yps.tile([N, DIM], f32)

        def emit_experts():
            for g in range(n_grp):
                w_t = wp.tile([128, grpw], mdt)
                csz = grpw // wsplit
                for ci in range(wsplit):
                    eng = getattr(nc, rings[ci % len(rings)])
                    eng.dma_start(w_t[:, ci * csz : (ci + 1) * csz],
                                  wpk_d[g][:, ci * csz : (ci + 1) * csz])
                b1_t = b1p.tile([1, group * H], mdt)
                getattr(nc, b1eng).dma_start(
                    b1_t[:].rearrange("o (i h) -> o i h", i=group),
                    b1_d[group * g : group * (g + 1), :].rearrange(
                        "(o i) h -> o i h", o=1),
                )
                for i in range(group):
                    e = group * g + i
                    h_ps = hps.tile([N, H], f32)
                    for s in range(NSEG):
                        seg = slice(s * 512, (s + 1) * 512)
                        for k in range(KD):
                            nc.tensor.matmul(
                                h_ps[:, seg],
                                lhsT=xT_sb[:, k * N : (k + 1) * N],
                                rhs=w_t[:, i * W1B + k * H + s * 512 :
                                        i * W1B + k * H + s * 512 + 512],
                                start=(k == 0), stop=False,
                            )
                        nc.tensor.matmul(
                            h_ps[:, seg],
                            lhsT=ones_sb[:],
                            rhs=b1_t[:, i * H + s * 512 : i * H + (s + 1) * 512],
                            start=False, stop=True,
                        )

                    hg = hgp.tile([N, H], f32)
                    nc.scalar.activation(hg[:], h_ps[:], GELU)
                    nc.vector.tensor_scalar_mul(hg[:], hg[:], cc_sb[:, e : e + 1])

                    hT_ps = hTps.tile([128, H], f32)
                    for j in range(KH):
                        nc.tensor.transpose(
                            hT_ps[:, j * 128 : (j + 1) * 128],
                            hg[:, j * 128 : (j + 1) * 128],
                            id_sb[:],
                        )
                    hT_sb = hTsp.tile([128, H], mdt)
                    nc.vector.tensor_copy(hT_sb[:], hT_ps[:])

                    w2off = group * W1B + i * W2B
                    for j in range(KH):
                        nc.tensor.matmul(
                            y_ps[:],
                            lhsT=hT_sb[:, j * 128 : (j + 1) * 128],
                            rhs=w_t[:, w2off + j * DIM : w2off + (j + 1) * DIM],
                            start=(e == 0 and j == 0), stop=False,
                        )

        if repeat > 1:
            # timing-only variant: re-run the whole expert sweep on-device
            # to amortize host/tunnel dispatch overhead. hint_engines arms
            # back-edge branch prefetch for the >256-inst PE/sync bodies so
            # the loop edge costs ~0.3us instead of a ~4us IRAM refetch.
            hint = (mybir.EngineType.PE, mybir.EngineType.SP)
            with tc.For_i(0, repeat, 1, hint_engines=hint):
                emit_experts()
        else:
            emit_experts()

        nc.tensor.matmul(
            y_ps[:], lhsT=ct_sb[:], rhs=b2_sb[:],
            start=False, stop=True,
        )
        o_sb = outp.tile([N, DIM], f32)
        nc.vector.tensor_copy(o_sb[:], y_ps[:])
        nc.sync.dma_start(out_d[:, :], o_sb[:])

    nc.compile()
    return nc


def get_program(mm_dtype_name=MM_DTYPE, act="Gelu", n_pairs=GP, repeat=1,
                wsplit=4, rings=("sync",), group=2, wbufs=2,
                b1eng="gpsimd"):
    key = (mm_dtype_name, act, n_pairs, repeat, wsplit, tuple(rings), group,
           wbufs, b1eng)
    if key not in _prog_cache:
        _prog_cache[key] = _build_program(mm_dtype_name, act, n_pairs, repeat,
                                          wsplit, rings, group, wbufs, b1eng)
    return _prog_cache[key]


def _softmax(v, axis=-1):
    m = np.max(v, axis=axis, keepdims=True)
    ex = np.exp(v - m)
    return ex / np.sum(ex, axis=axis, keepdims=True)


def host_routing(x, router_w, router_b):
    """Replicates the reference routing in fp32 numpy: softmax over all
    experts, take top-K probs, renormalize those with another softmax."""
    xt = np.asarray(x, np.float32).reshape(N, DIM)
    logits = xt @ np.asarray(router_w, np.float32) + np.asarray(router_b, np.float32)
    probs = _softmax(logits, axis=-1)
    idx = np.argpartition(probs, E - K, axis=-1)[:, E - K:]          # top-K set
    vals = np.take_along_axis(probs, idx, axis=-1)
    w = _softmax(vals, axis=-1)
    comb = np.zeros((N, E), np.float32)
    np.put_along_axis(comb, idx, w.astype(np.float32), axis=-1)
    return comb


def pack_weights(w1c, w2c, group=2):
    """[32,512,1024] + [32,1024,512] -> [32/group, 128, group*12288]:
    per expert group, per partition, [w1(i,k,h) | w2(i,k,d)] contiguous."""
    ng = EPC // group
    a = (w1c.reshape(ng, group, KD_, 128, H).transpose(0, 3, 1, 2, 4)
         .reshape(ng, 128, group * W1B))
    b = (w2c.reshape(ng, group, KH_, 128, DIM).transpose(0, 3, 1, 2, 4)
         .reshape(ng, 128, group * W2B))
    return np.ascontiguousarray(np.concatenate([a, b], axis=2))


KD_ = DIM // 128
KH_ = H // 128


def make_in_maps(x, w1, b1, w2, b2, router_w, router_b, group=2):
    x = np.ascontiguousarray(np.asarray(x, np.float32))
    w1 = np.asarray(w1, np.float32)
    b1 = np.asarray(b1, np.float32)
    w2 = np.asarray(w2, np.float32)
    b2 = np.asarray(b2, np.float32)
    comb = host_routing(x, router_w, router_b)
    xT = np.ascontiguousarray(x.reshape(N, DIM).T)
    ident = np.eye(128, dtype=np.float32)
    in_maps = []
    for c in range(N_CORES):
        sl = slice(c * EPC, (c + 1) * EPC)
        cl = np.ascontiguousarray(comb[:, sl])
        in_maps.append({
            "xT": xT,
            "wpk": pack_weights(w1[sl], w2[sl], group),
            "b1s": np.ascontiguousarray(b1[sl]),
            "b2s": np.ascontiguousarray(b2[sl]),
            "combc": cl,
            "combT": np.ascontiguousarray(cl.T),
            "ident": ident,
            "ones": np.ones((1, N), np.float32),
        })
    return in_maps


def kernel(x, w1, b1, w2, b2, router_w, router_b):
    from concourse.bass_utils import run_bass_kernel_spmd

    nc = get_program()
    in_maps = make_in_maps(x, w1, b1, w2, b2, router_w, router_b)
    res = run_bass_kernel_spmd(nc, in_maps, list(range(N_CORES)))
    out = np.zeros((N, DIM), np.float32)
    for r in res.results:
        out += r["out"]
    return out.reshape(B, T, DIM).astype(np.float32)
